# revision 4
# baseline (speedup 1.0000x reference)
import sys
sys.path.insert(0, "/opt/trn_rl_repo")
import numpy as np
import concourse.bass as bass
import concourse.tile as tile
from concourse import bacc, mybir
from concourse.bass_utils import run_bass_kernel_spmd
from concourse.masks import make_identity

F32 = mybir.dt.float32
F32R = mybir.dt.float32r
U32 = mybir.dt.uint32
AX = mybir.AxisListType.X
EXP = mybir.ActivationFunctionType.Exp

KSIZE, STRIDE, RATE, SCALE = 3, 1, 2, 10.0
IH = IW = 48
L = IH * IW          # 2304
CS = 128

# ---------------- host prep ----------------

def _extract_patches(img, kernel, stride=1):
    bsz, c, h, w = img.shape
    h2 = -(-h // stride)
    w2 = -(-w // stride)
    pad_row = (h2 - 1) * stride + kernel - h
    pad_col = (w2 - 1) * stride + kernel - w
    x = np.pad(img, ((0, 0), (0, 0),
                     (pad_col // 2, pad_col - pad_col // 2),
                     (pad_row // 2, pad_row - pad_row // 2)))
    cols = []
    for i in range(kernel):
        for j in range(kernel):
            cols.append(x[:, :, i:i + (h2 - 1) * stride + 1:stride,
                          j:j + (w2 - 1) * stride + 1:stride])
    p = np.stack(cols, axis=-1).reshape(bsz, c, h2, w2, kernel, kernel)
    p = p.transpose(0, 4, 5, 1, 2, 3)
    return np.ascontiguousarray(p).reshape(bsz, -1, h2, w2)


def _round_fp32r(x):
    b = np.ascontiguousarray(x, dtype=np.float32).view(np.uint32)
    r = ((b.astype(np.uint64) + 0x800) & 0xFFFFF000).astype(np.uint32)
    return r.view(np.float32)


def _prep_sample(f_s, b_s, mask_s):
    kernel = RATE * 2
    fd = f_s[:, ::RATE, ::RATE]
    bd = b_s[:, ::RATE, ::RATE]
    md = mask_s[:, ::RATE, ::RATE]
    wp = _extract_patches(bd[None], KSIZE, STRIDE)
    wi = wp.reshape(1, -1, KSIZE, KSIZE, CS).transpose(0, 1, 4, 2, 3)[0]
    norm = np.sqrt(np.sum(wi * wi, axis=(1, 2, 3), dtype=np.float32) + np.float32(1e-8))
    wi_n = (wi / np.maximum(norm, np.float32(1e-4))[:, None, None, None]).astype(np.float32)
    wnt = np.ascontiguousarray(wi_n.transpose(2, 3, 1, 0).reshape(9, CS, L))
    mp = _extract_patches(md[None], KSIZE, STRIDE)
    m = mp.reshape(1, -1, 1, KSIZE, KSIZE).transpose(0, 2, 3, 4, 1)[0]
    mm = (np.mean(1.0 - m, axis=(0, 1, 2)) < 0.85).astype(np.float32)
    rp = _extract_patches(b_s[None], kernel, RATE * STRIDE)
    raw_wi = rp.reshape(1, -1, kernel, kernel, CS).transpose(0, 1, 4, 2, 3)[0]
    rawt = raw_wi.transpose(2, 3, 0, 1).reshape(16, 18, 128, CS) / np.float32(4.0)
    rawt = np.ascontiguousarray(_round_fp32r(rawt))
    fdp = np.zeros((CS, IH + 2, IW + 2), np.float32)
    fdp[:, 1:-1, 1:-1] = fd
    xc = np.zeros((9, CS, L), np.float32)
    ps = np.arange(L)
    fh, fw = ps // IW, ps % IW
    for di in range(3):
        for dj in range(3):
            xc[di * 3 + dj] = fdp[:, fh + di, fw + dj]
    return wnt, mm, rawt, xc


def _wnt_window(wnt, a_lo, n):
    out = np.zeros((9, CS, n), np.float32)
    a = np.arange(a_lo, a_lo + n)
    v = (a >= 0) & (a < L)
    out[:, :, v] = wnt[:, :, a[v]]
    return np.ascontiguousarray(out)


# ---------------- device program ----------------

_CACHED = {}


def _build_program():
    if "nc" in _CACHED:
        return _CACHED["nc"]
    nc = bacc.Bacc(None, target_bir_lowering=False, debug=False, num_devices=8)
    wl_d = nc.dram_tensor("wl", [9, 128, 1408], F32, kind="ExternalInput").ap()
    wf_d = nc.dram_tensor("wf", [9, 128, 128], F32, kind="ExternalInput").ap()
    xc_d = nc.dram_tensor("xc", [9, 128, L], F32, kind="ExternalInput").ap()
    mm10_d = nc.dram_tensor("mm10", [1, L], F32, kind="ExternalInput").ap()
    mmv_d = nc.dram_tensor("mmv", [1, L], F32, kind="ExternalInput").ap()
    gates_d = nc.dram_tensor("gates", [128, 4], F32, kind="ExternalInput").ap()
    rawt_d = nc.dram_tensor("rawt", [16, 18, 128, 128], F32R, kind="ExternalInput").ap()
    ysl_d = nc.dram_tensor("ysl", [128, 50 * 96], F32, kind="ExternalOutput").ap()
    idx_d = nc.dram_tensor("idx", [9, 128, 8], U32, kind="ExternalOutput").ap()
    zs = nc.dram_tensor("zs", [11, 128, L], F32).ap()
    zfs = nc.dram_tensor("zfs", [128, L], F32).ap()
    z1s = nc.dram_tensor("z1s", [11, 128, L], F32).ap()
    yts = nc.dram_tensor("yts", [18, 128, 1152], F32R).ap()
    zs_rows = zs.rearrange("t p l -> (t p) l")       # row-addressable
    z1_rows = z1s.rearrange("t p l -> (t p) l")
    PCH = [(0, 512), (512, 512), (1024, 512), (1536, 512), (2048, 256)]

    with tile.TileContext(nc) as tc:
        with tc.tile_pool(name="const", bufs=1) as cpool:
            ident = cpool.tile([128, 128], F32)
            make_identity(nc, ident)
            gates = cpool.tile([128, 4], F32)
            nc.sync.dma_start(out=gates, in_=gates_d)

            # ---------- phase A: conv1 ----------
            with tc.tile_pool(name="pa", bufs=1) as pa, \
                 tc.tile_pool(name="pa2", bufs=2) as pa2, \
                 tc.tile_pool(name="paps", bufs=1, space="PSUM") as paps:
                wl_sb = pa.tile([128, 9, 1408], F32)
                nc.sync.dma_start(out=wl_sb, in_=wl_d.rearrange("k c a -> c k a"))
                wf_sb = pa.tile([128, 9, 128], F32)
                nc.sync.dma_start(out=wf_sb, in_=wf_d.rearrange("k c a -> c k a"))
                xc_sb = pa.tile([128, 9, L], F32)
                nc.sync.dma_start(out=xc_sb, in_=xc_d.rearrange("k c p -> c k p"))
                for slot in range(12):
                    far = slot == 11
                    zt = pa2.tile([128, L], F32, tag="ztile")
                    accs = [paps.tile([128, n], F32, tag=f"ps{i}", name=f"ps{i}")
                            for i, (_, n) in enumerate(PCH)]
                    for k in range(9):
                        lhs = wf_sb[:, k, :] if far else wl_sb[:, k, 128 * slot:128 * slot + 128]
                        for i, (p0, n) in enumerate(PCH):
                            nc.tensor.matmul(out=accs[i], lhsT=lhs,
                                             rhs=xc_sb[:, k, p0:p0 + n],
                                             start=(k == 0), stop=(k == 8))
                    for i, (p0, n) in enumerate(PCH):
                        nc.vector.tensor_copy(out=zt[:, p0:p0 + n], in_=accs[i])
                    nc.sync.dma_start(out=(zfs if far else zs[slot]), in_=zt)

            # ---------- phase B: fuse + softmax + transpose ----------
            with tc.tile_pool(name="pb", bufs=1) as pb, \
                 tc.tile_pool(name="pbl", bufs=4) as pbl, \
                 tc.tile_pool(name="pb1", bufs=1) as pb1, \
                 tc.tile_pool(name="pb3", bufs=3) as pb3, \
                 tc.tile_pool(name="pb2", bufs=2) as pb2, \
                 tc.tile_pool(name="pbs", bufs=2) as pbs, \
                 tc.tile_pool(name="pbps", bufs=4, space="PSUM") as pbps:
                mm10r = pb.tile([128, L], F32)
                nc.sync.dma_start(out=mm10r, in_=bass.AP(tensor=mm10_d.tensor, offset=0,
                                                         ap=[[0, 128], [1, L]]))
                mmr = pb.tile([128, L], F32)
                nc.sync.dma_start(out=mmr, in_=bass.AP(tensor=mmv_d.tensor, offset=0,
                                                       ap=[[0, 128], [1, L]]))

                def f1(zc, zp1, zm1, outt):
                    t1 = pb2.tile([128, L], F32, tag="f1tmp")
                    nc.vector.tensor_add(t1[:, 0:2303], zc[:, 0:2303], zp1[:, 1:2304])
                    nc.vector.tensor_copy(out=t1[:, 2303:2304], in_=zc[:, 2303:2304])
                    nc.vector.tensor_add(outt[:, 1:2304], t1[:, 1:2304], zm1[:, 0:2303])
                    nc.vector.tensor_copy(out=outt[:, 0:1], in_=t1[:, 0:1])

                # far z1
                zfc = pbl.tile([128, L], F32, tag="zload")
                nc.sync.dma_start(out=zfc, in_=zfs)
                zfp = pbl.tile([128, L], F32, tag="zload")
                nc.vector.memset(zfp, 0.0)
                nc.sync.dma_start(out=zfp[0:127, :], in_=zfs[1:128, :])
                zfm = pbl.tile([128, L], F32, tag="zload")
                nc.vector.memset(zfm, 0.0)
                nc.sync.dma_start(out=zfm[1:128, :], in_=zfs[0:127, :])
                z1f = pb1.tile([128, L], F32)
                f1(zfc, zfp, zfm, z1f)
                Em = pb1.tile([128, L], F32)
                nc.vector.memset(Em, 0.0)
                nc.sync.dma_start(out=Em[1:48, :], in_=z1f[80:127, :])
                nc.vector.tensor_scalar_mul(Em, Em, gates[:, 0:1])
                Ep = pb1.tile([128, L], F32)
                nc.vector.memset(Ep, 0.0)
                nc.sync.dma_start(out=Ep[80:127, :], in_=z1f[1:48, :])
                nc.vector.tensor_scalar_mul(Ep, Ep, gates[:, 1:2])

                # boundary partial z1 tiles (slot0 gated by col2, slot10 by col3)
                for base, gcol in ((0, 2), (1280, 3)):
                    zc = pbl.tile([128, L], F32, tag="zload")
                    nc.sync.dma_start(out=zc, in_=zs_rows[base:base + 128, :])
                    zp1 = pbl.tile([128, L], F32, tag="zload")
                    if base == 1280:
                        nc.vector.memset(zp1, 0.0)
                        nc.sync.dma_start(out=zp1[0:127, :], in_=zs_rows[1281:1408, :])
                    else:
                        nc.sync.dma_start(out=zp1, in_=zs_rows[base + 1:base + 129, :])
                    zm1 = pbl.tile([128, L], F32, tag="zload")
                    if base == 0:
                        nc.vector.memset(zm1, 0.0)
                        nc.sync.dma_start(out=zm1[1:128, :], in_=zs_rows[0:127, :])
                    else:
                        nc.sync.dma_start(out=zm1, in_=zs_rows[base - 1:base + 127, :])
                    z1t = pb3.tile([128, L], F32, tag="z1t")
                    f1(zc, zp1, zm1, z1t)
                    nc.vector.tensor_scalar_mul(z1t, z1t, gates[:, gcol:gcol + 1])
                    nc.sync.dma_start(out=z1_rows[base:base + 128, :], in_=z1t)

                z1own = {}
                for so in range(1, 10):
                    r0 = 128 * so
                    zc = pbl.tile([128, L], F32, tag="zload")
                    nc.sync.dma_start(out=zc, in_=zs_rows[r0:r0 + 128, :])
                    zp1 = pbl.tile([128, L], F32, tag="zload")
                    nc.sync.dma_start(out=zp1, in_=zs_rows[r0 + 1:r0 + 129, :])
                    zm1 = pbl.tile([128, L], F32, tag="zload")
                    nc.sync.dma_start(out=zm1, in_=zs_rows[r0 - 1:r0 + 127, :])
                    z1t = pb3.tile([128, L], F32, tag="z1t")
                    f1(zc, zp1, zm1, z1t)
                    nc.sync.dma_start(out=z1_rows[r0:r0 + 128, :], in_=z1t)
                    z1own[so] = z1t

                for so in range(1, 10):
                    r0 = 128 * so
                    z1c = z1own[so]
                    zp48 = pb2.tile([128, L], F32, tag="zp48")
                    nc.sync.dma_start(out=zp48, in_=z1_rows[r0 + 48:r0 + 176, :])
                    zm48 = pb2.tile([128, L], F32, tag="zm48")
                    nc.sync.dma_start(out=zm48, in_=z1_rows[r0 - 48:r0 + 80, :])
                    z2 = pb2.tile([128, L], F32, tag="z2")
                    nc.vector.tensor_add(z2[:, 0:2256], z1c[:, 0:2256], zp48[:, 48:2304])
                    nc.vector.tensor_copy(out=z2[:, 2256:2304], in_=z1c[:, 2256:2304])
                    nc.vector.tensor_add(z2[:, 2256:2303], z2[:, 2256:2303], zp48[:, 1:48])
                    nc.vector.tensor_add(z2[:, 48:2304], z2[:, 48:2304], zm48[:, 0:2256])
                    nc.vector.tensor_add(z2[:, 1:48], z2[:, 1:48], zm48[:, 2256:2303])
                    if so == 1:
                        nc.vector.tensor_add(z2[:, 48:2304], z2[:, 48:2304], Em[:, 0:2256])
                        nc.vector.tensor_add(z2[:, 1:48], z2[:, 1:48], Em[:, 2256:2303])
                    if so == 9:
                        nc.vector.tensor_add(z2[:, 0:2256], z2[:, 0:2256], Ep[:, 48:2304])
                        nc.vector.tensor_add(z2[:, 2256:2303], z2[:, 2256:2303], Ep[:, 1:48])
                    nc.vector.tensor_mul(z2, z2, mm10r)
                    negmax = pbs.tile([128, 1], F32, tag="nm")
                    nc.vector.reduce_max(out=negmax, in_=z2, axis=AX, negate=True)
                    et = pb2.tile([128, L], F32, tag="et")
                    ssum = pbs.tile([128, 1], F32, tag="ss")
                    nc.scalar.activation(out=et, in_=z2, func=EXP, bias=negmax,
                                         accum_out=ssum)
                    rinv = pbs.tile([128, 1], F32, tag="ri")
                    nc.vector.reciprocal(rinv, ssum)
                    nc.vector.tensor_scalar_mul(et, et, rinv)
                    nc.vector.tensor_mul(et, et, mmr)
                    mx8 = pbs.tile([128, 8], F32, tag="mx")
                    idxt = pbs.tile([128, 8], U32, tag="ix")
                    nc.vector.max(mx8, et)
                    nc.vector.max_index(idxt, mx8, et)
                    nc.sync.dma_start(out=idx_d[so - 1], in_=idxt)
                    for bc in range(18):
                        pt = pbps.tile([128, 128], F32, tag="tp")
                        nc.tensor.transpose(pt, et[:, 128 * bc:128 * bc + 128], ident)
                        yst = pbs.tile([128, 128], F32R, tag="yst")
                        nc.any.tensor_copy(out=yst, in_=pt)
                        nc.sync.dma_start(
                            out=yts[bc][:, 128 * (so - 1):128 * (so - 1) + 128],
                            in_=yst)

            # ---------- phase C: conv transpose ----------
            with tc.tile_pool(name="pc1", bufs=1) as pc1, \
                 tc.tile_pool(name="pc2", bufs=2) as pc2, \
                 tc.tile_pool(name="pcps", bufs=6, space="PSUM") as pcps:
                ysb = pc1.tile([128, 18, 1152], F32R)
                nc.sync.dma_start(out=ysb, in_=yts.rearrange("t b a -> b t a"))
                slab = pc1.tile([128, 50 * 96], F32)
                nc.vector.memset(slab, 0.0)
                for tap in range(16):
                    ky, kx = tap // 4, tap % 4
                    raws = pc2.tile([128, 18, 128], F32R, tag="raws")
                    nc.sync.dma_start(out=raws, in_=rawt_d[tap].rearrange("t b c -> b t c"))
                    fw0 = 1 if kx == 0 else 0
                    fw1 = 47 if kx == 3 else 48
                    nfw = fw1 - fw0
                    for pc in range(3):
                        acc = pcps.tile([128, 384], F32, tag="ct")
                        for bc in range(18):
                            nc.tensor.matmul(out=acc, lhsT=raws[:, bc, :],
                                             rhs=ysb[:, bc, 384 * pc:384 * pc + 384],
                                             start=(bc == 0), stop=(bc == 17))
                        dst = bass.AP(tensor=slab.tensor, offset=slab.offset
                                      + ((16 * pc + ky) * 96 + (kx - 1) + 2 * fw0),
                                      ap=[slab.ap[0], [192, 8], [2, nfw]])
                        src = bass.AP(tensor=acc.tensor, offset=acc.offset + fw0,
                                      ap=[acc.ap[0], [48, 8], [1, nfw]])
                        nc.vector.tensor_add(dst, dst, src)
                nc.sync.dma_start(out=ysl_d, in_=slab)

    nc.compile()
    _CACHED["nc"] = nc
    return nc


def kernel(f, b, mask):
    f = np.asarray(f, np.float32)
    b = np.asarray(b, np.float32)
    mask = np.asarray(mask, np.float32)
    bs = f.shape[0]
    nc = _build_program()
    in_maps = []
    metas = []
    for s in range(bs):
        wnt, mm, rawt, xc = _prep_sample(f[s], b[s], mask[s])
        mm10 = (mm * np.float32(SCALE)).astype(np.float32)[None, :]
        for h in (0, 1):
            wl = _wnt_window(wnt, (-1 + 9 * h) * 128, 1408)
            wf = _wnt_window(wnt, 2176 if h == 0 else 0, 128)
            g = np.zeros((128, 4), np.float32)
            g[:, 0] = 1.0 - h
            g[:, 1] = h
            g[:, 2] = h
            g[:, 3] = 1.0 - h
            in_maps.append(dict(wl=wl, wf=wf, xc=xc, mm10=mm10,
                                mmv=mm[None, :].astype(np.float32),
                                gates=g, rawt=rawt))
            metas.append((s, h))
    r = run_bass_kernel_spmd(nc, in_maps, list(range(8)))
    _CACHED["last"] = (nc, in_maps)

    y = np.zeros((bs, CS, 96, 96), np.float32)
    off_flat = np.zeros((bs, L), np.int64)
    for ci, (s, h) in enumerate(metas):
        res = r.results[ci]
        slab = res["ysl"].reshape(CS, 50, 96)
        if h == 0:
            y[s][:, 0:49, :] += slab[:, 1:50, :]
        else:
            y[s][:, 47:96, :] += slab[:, 0:49, :]
        idx = res["idx"][:, :, 0].astype(np.int64).reshape(9 * 128)
        off_flat[s, 1152 * h:1152 * h + 1152] = idx
    gi = np.arange(IH)[:, None] * np.ones((1, IW), np.int64)
    gj = np.ones((IH, 1), np.int64) * np.arange(IW)[None, :]
    offsets = np.stack([off_flat // 96 - gi.reshape(-1)[None, :],
                        off_flat % 96 - gj.reshape(-1)[None, :]], axis=1)
    offsets = offsets.reshape(bs, 2, IH, IW).astype(np.int32)
    return y, offsets


# revision 5
# speedup vs baseline: 1.4217x; 1.4217x over previous
import sys
sys.path.insert(0, "/opt/trn_rl_repo")
import numpy as np
import concourse.bass as bass
import concourse.tile as tile
from concourse import bacc, mybir
from concourse.bass_utils import run_bass_kernel_spmd
from concourse.masks import make_identity

F32 = mybir.dt.float32
F32R = mybir.dt.float32r
U32 = mybir.dt.uint32
AX = mybir.AxisListType.X
EXP = mybir.ActivationFunctionType.Exp

KSIZE, STRIDE, RATE, SCALE = 3, 1, 2, 10.0
IH = IW = 48
L = IH * IW          # 2304
CS = 128

# ---------------- host prep ----------------

def _extract_patches(img, kernel, stride=1):
    bsz, c, h, w = img.shape
    h2 = -(-h // stride)
    w2 = -(-w // stride)
    pad_row = (h2 - 1) * stride + kernel - h
    pad_col = (w2 - 1) * stride + kernel - w
    x = np.pad(img, ((0, 0), (0, 0),
                     (pad_col // 2, pad_col - pad_col // 2),
                     (pad_row // 2, pad_row - pad_row // 2)))
    cols = []
    for i in range(kernel):
        for j in range(kernel):
            cols.append(x[:, :, i:i + (h2 - 1) * stride + 1:stride,
                          j:j + (w2 - 1) * stride + 1:stride])
    p = np.stack(cols, axis=-1).reshape(bsz, c, h2, w2, kernel, kernel)
    p = p.transpose(0, 4, 5, 1, 2, 3)
    return np.ascontiguousarray(p).reshape(bsz, -1, h2, w2)


def _round_fp32r(x):
    b = np.ascontiguousarray(x, dtype=np.float32).view(np.uint32)
    r = ((b.astype(np.uint64) + 0x800) & 0xFFFFF000).astype(np.uint32)
    return r.view(np.float32)


def _prep_sample(f_s, b_s, mask_s):
    kernel = RATE * 2
    fd = f_s[:, ::RATE, ::RATE]
    bd = b_s[:, ::RATE, ::RATE]
    md = mask_s[:, ::RATE, ::RATE]
    wp = _extract_patches(bd[None], KSIZE, STRIDE)
    wi = wp.reshape(1, -1, KSIZE, KSIZE, CS).transpose(0, 1, 4, 2, 3)[0]
    norm = np.sqrt(np.sum(wi * wi, axis=(1, 2, 3), dtype=np.float32) + np.float32(1e-8))
    wi_n = (wi / np.maximum(norm, np.float32(1e-4))[:, None, None, None]).astype(np.float32)
    wnt = np.ascontiguousarray(wi_n.transpose(2, 3, 1, 0).reshape(9, CS, L))
    mp = _extract_patches(md[None], KSIZE, STRIDE)
    m = mp.reshape(1, -1, 1, KSIZE, KSIZE).transpose(0, 2, 3, 4, 1)[0]
    mm = (np.mean(1.0 - m, axis=(0, 1, 2)) < 0.85).astype(np.float32)
    rp = _extract_patches(b_s[None], kernel, RATE * STRIDE)
    raw_wi = rp.reshape(1, -1, kernel, kernel, CS).transpose(0, 1, 4, 2, 3)[0]
    rawt = raw_wi.transpose(2, 3, 0, 1).reshape(16, 18, 128, CS) / np.float32(4.0)
    rawt = np.ascontiguousarray(_round_fp32r(rawt))
    fdp = np.zeros((CS, IH + 2, IW + 2), np.float32)
    fdp[:, 1:-1, 1:-1] = fd
    return wnt, mm, rawt, fdp.reshape(CS, -1)


def _wnt_window(wnt, a_lo, n):
    out = np.zeros((9, CS, n), np.float32)
    a = np.arange(a_lo, a_lo + n)
    v = (a >= 0) & (a < L)
    out[:, :, v] = wnt[:, :, a[v]]
    return np.ascontiguousarray(out)


# ---------------- device program ----------------

_CACHED = {}


def _build_program():
    if "nc" in _CACHED:
        return _CACHED["nc"]
    nc = bacc.Bacc(None, target_bir_lowering=False, debug=False, num_devices=8)
    wl_d = nc.dram_tensor("wl", [9, 128, 1408], F32, kind="ExternalInput").ap()
    wf_d = nc.dram_tensor("wf", [9, 128, 128], F32, kind="ExternalInput").ap()
    fdp_d = nc.dram_tensor("fdp", [128, 2500], F32, kind="ExternalInput").ap()
    mm10_d = nc.dram_tensor("mm10", [1, L], F32, kind="ExternalInput").ap()
    mmv_d = nc.dram_tensor("mmv", [1, L], F32, kind="ExternalInput").ap()
    gates_d = nc.dram_tensor("gates", [128, 4], F32, kind="ExternalInput").ap()
    rawt_d = nc.dram_tensor("rawt", [16, 18, 128, 128], F32R, kind="ExternalInput").ap()
    ysl_d = nc.dram_tensor("ysl", [128, 50 * 96], F32, kind="ExternalOutput").ap()
    idx_d = nc.dram_tensor("idx", [9, 128, 8], U32, kind="ExternalOutput").ap()
    zs = nc.dram_tensor("zs", [11, 128, L], F32).ap()
    zfs = nc.dram_tensor("zfs", [128, L], F32).ap()
    z1s = nc.dram_tensor("z1s", [11, 128, L], F32).ap()
    yts = nc.dram_tensor("yts", [18, 128, 1152], F32R).ap()
    zs_rows = zs.rearrange("t p l -> (t p) l")       # row-addressable
    z1_rows = z1s.rearrange("t p l -> (t p) l")
    PCH = [(0, 512), (512, 512), (1024, 512), (1536, 512), (2048, 256)]

    with tile.TileContext(nc) as tc:
        with tc.tile_pool(name="const", bufs=1) as cpool:
            ident = cpool.tile([128, 128], F32)
            make_identity(nc, ident)
            gates = cpool.tile([128, 4], F32)
            nc.sync.dma_start(out=gates, in_=gates_d)

            # ---------- phase A: conv1 ----------
            with tc.tile_pool(name="pa", bufs=1) as pa, \
                 tc.tile_pool(name="pa2", bufs=2) as pa2, \
                 tc.tile_pool(name="paps", bufs=1, space="PSUM") as paps:
                wl_sb = pa.tile([128, 9, 1408], F32)
                nc.sync.dma_start(out=wl_sb, in_=wl_d.rearrange("k c a -> c k a"))
                wf_sb = pa.tile([128, 9, 128], F32)
                nc.sync.dma_start(out=wf_sb, in_=wf_d.rearrange("k c a -> c k a"))
                fdp_sb = pa.tile([128, 2500], F32)
                nc.sync.dma_start(out=fdp_sb, in_=fdp_d)
                xc_sb = pa.tile([128, 9, L], F32)
                for di in range(3):
                    for dj in range(3):
                        src = bass.AP(tensor=fdp_sb.tensor,
                                      offset=fdp_sb.offset + di * 50 + dj,
                                      ap=[fdp_sb.ap[0], [50, 48], [1, 48]])
                        nc.vector.tensor_copy(out=xc_sb[:, di * 3 + dj, :], in_=src)
                for slot in range(12):
                    far = slot == 11
                    zt = pa2.tile([128, L], F32, tag="ztile")
                    accs = [paps.tile([128, n], F32, tag=f"ps{i}", name=f"ps{i}")
                            for i, (_, n) in enumerate(PCH)]
                    for k in range(9):
                        lhs = wf_sb[:, k, :] if far else wl_sb[:, k, 128 * slot:128 * slot + 128]
                        for i, (p0, n) in enumerate(PCH):
                            nc.tensor.matmul(out=accs[i], lhsT=lhs,
                                             rhs=xc_sb[:, k, p0:p0 + n],
                                             start=(k == 0), stop=(k == 8))
                    for i, (p0, n) in enumerate(PCH):
                        nc.vector.tensor_copy(out=zt[:, p0:p0 + n], in_=accs[i])
                    nc.sync.dma_start(out=(zfs if far else zs[slot]), in_=zt)

            # ---------- phase B: fuse + softmax + transpose ----------
            with tc.tile_pool(name="pb", bufs=1) as pb, \
                 tc.tile_pool(name="pbl", bufs=4) as pbl, \
                 tc.tile_pool(name="pb1", bufs=1) as pb1, \
                 tc.tile_pool(name="pb3", bufs=3) as pb3, \
                 tc.tile_pool(name="pb2", bufs=2) as pb2, \
                 tc.tile_pool(name="pbs", bufs=2) as pbs, \
                 tc.tile_pool(name="pbps", bufs=4, space="PSUM") as pbps:
                mm10r = pb.tile([128, L], F32)
                nc.sync.dma_start(out=mm10r, in_=bass.AP(tensor=mm10_d.tensor, offset=0,
                                                         ap=[[0, 128], [1, L]]))
                mmr = pb.tile([128, L], F32)
                nc.sync.dma_start(out=mmr, in_=bass.AP(tensor=mmv_d.tensor, offset=0,
                                                       ap=[[0, 128], [1, L]]))

                def f1(zc, zp1, zm1, outt):
                    t1 = pb2.tile([128, L], F32, tag="f1tmp")
                    nc.vector.tensor_add(t1[:, 0:2303], zc[:, 0:2303], zp1[:, 1:2304])
                    nc.vector.tensor_copy(out=t1[:, 2303:2304], in_=zc[:, 2303:2304])
                    nc.vector.tensor_add(outt[:, 1:2304], t1[:, 1:2304], zm1[:, 0:2303])
                    nc.vector.tensor_copy(out=outt[:, 0:1], in_=t1[:, 0:1])

                # far z1
                zfc = pbl.tile([128, L], F32, tag="zload")
                nc.sync.dma_start(out=zfc, in_=zfs)
                zfp = pbl.tile([128, L], F32, tag="zload")
                nc.vector.memset(zfp, 0.0)
                nc.sync.dma_start(out=zfp[0:127, :], in_=zfs[1:128, :])
                zfm = pbl.tile([128, L], F32, tag="zload")
                nc.vector.memset(zfm, 0.0)
                nc.sync.dma_start(out=zfm[1:128, :], in_=zfs[0:127, :])
                z1f = pb1.tile([128, L], F32)
                f1(zfc, zfp, zfm, z1f)
                Em = pb1.tile([128, L], F32)
                nc.vector.memset(Em, 0.0)
                nc.sync.dma_start(out=Em[1:48, :], in_=z1f[80:127, :])
                nc.vector.tensor_scalar_mul(Em, Em, gates[:, 0:1])
                Ep = pb1.tile([128, L], F32)
                nc.vector.memset(Ep, 0.0)
                nc.sync.dma_start(out=Ep[80:127, :], in_=z1f[1:48, :])
                nc.vector.tensor_scalar_mul(Ep, Ep, gates[:, 1:2])

                # boundary partial z1 tiles (slot0 gated by col2, slot10 by col3)
                for base, gcol in ((0, 2), (1280, 3)):
                    zc = pbl.tile([128, L], F32, tag="zload")
                    nc.sync.dma_start(out=zc, in_=zs_rows[base:base + 128, :])
                    zp1 = pbl.tile([128, L], F32, tag="zload")
                    if base == 1280:
                        nc.vector.memset(zp1, 0.0)
                        nc.sync.dma_start(out=zp1[0:127, :], in_=zs_rows[1281:1408, :])
                    else:
                        nc.sync.dma_start(out=zp1, in_=zs_rows[base + 1:base + 129, :])
                    zm1 = pbl.tile([128, L], F32, tag="zload")
                    if base == 0:
                        nc.vector.memset(zm1, 0.0)
                        nc.sync.dma_start(out=zm1[1:128, :], in_=zs_rows[0:127, :])
                    else:
                        nc.sync.dma_start(out=zm1, in_=zs_rows[base - 1:base + 127, :])
                    z1t = pb3.tile([128, L], F32, tag="z1t")
                    f1(zc, zp1, zm1, z1t)
                    nc.vector.tensor_scalar_mul(z1t, z1t, gates[:, gcol:gcol + 1])
                    nc.sync.dma_start(out=z1_rows[base:base + 128, :], in_=z1t)

                z1own = {}
                for so in range(1, 10):
                    r0 = 128 * so
                    zc = pbl.tile([128, L], F32, tag="zload")
                    nc.sync.dma_start(out=zc, in_=zs_rows[r0:r0 + 128, :])
                    zp1 = pbl.tile([128, L], F32, tag="zload")
                    nc.sync.dma_start(out=zp1, in_=zs_rows[r0 + 1:r0 + 129, :])
                    zm1 = pbl.tile([128, L], F32, tag="zload")
                    nc.sync.dma_start(out=zm1, in_=zs_rows[r0 - 1:r0 + 127, :])
                    z1t = pb3.tile([128, L], F32, tag="z1t")
                    f1(zc, zp1, zm1, z1t)
                    nc.sync.dma_start(out=z1_rows[r0:r0 + 128, :], in_=z1t)
                    z1own[so] = z1t

                for so in range(1, 10):
                    r0 = 128 * so
                    z1c = z1own[so]
                    zp48 = pb2.tile([128, L], F32, tag="zp48")
                    nc.sync.dma_start(out=zp48, in_=z1_rows[r0 + 48:r0 + 176, :])
                    zm48 = pb2.tile([128, L], F32, tag="zm48")
                    nc.sync.dma_start(out=zm48, in_=z1_rows[r0 - 48:r0 + 80, :])
                    z2 = pb2.tile([128, L], F32, tag="z2")
                    nc.vector.tensor_add(z2[:, 0:2256], z1c[:, 0:2256], zp48[:, 48:2304])
                    nc.vector.tensor_copy(out=z2[:, 2256:2304], in_=z1c[:, 2256:2304])
                    nc.vector.tensor_add(z2[:, 2256:2303], z2[:, 2256:2303], zp48[:, 1:48])
                    nc.vector.tensor_add(z2[:, 48:2304], z2[:, 48:2304], zm48[:, 0:2256])
                    nc.vector.tensor_add(z2[:, 1:48], z2[:, 1:48], zm48[:, 2256:2303])
                    if so == 1:
                        nc.vector.tensor_add(z2[:, 48:2304], z2[:, 48:2304], Em[:, 0:2256])
                        nc.vector.tensor_add(z2[:, 1:48], z2[:, 1:48], Em[:, 2256:2303])
                    if so == 9:
                        nc.vector.tensor_add(z2[:, 0:2256], z2[:, 0:2256], Ep[:, 48:2304])
                        nc.vector.tensor_add(z2[:, 2256:2303], z2[:, 2256:2303], Ep[:, 1:48])
                    nc.vector.tensor_mul(z2, z2, mm10r)
                    negmax = pbs.tile([128, 1], F32, tag="nm")
                    nc.vector.reduce_max(out=negmax, in_=z2, axis=AX, negate=True)
                    et = pb2.tile([128, L], F32, tag="et")
                    ssum = pbs.tile([128, 1], F32, tag="ss")
                    nc.scalar.activation(out=et, in_=z2, func=EXP, bias=negmax,
                                         accum_out=ssum)
                    rinv = pbs.tile([128, 1], F32, tag="ri")
                    nc.vector.reciprocal(rinv, ssum)
                    nc.vector.tensor_scalar_mul(et, et, rinv)
                    nc.vector.tensor_mul(et, et, mmr)
                    mx8 = pbs.tile([128, 8], F32, tag="mx")
                    idxt = pbs.tile([128, 8], U32, tag="ix")
                    nc.vector.max(mx8, et)
                    nc.vector.max_index(idxt, mx8, et)
                    nc.sync.dma_start(out=idx_d[so - 1], in_=idxt)
                    for bc in range(18):
                        pt = pbps.tile([128, 128], F32, tag="tp")
                        nc.tensor.transpose(pt, et[:, 128 * bc:128 * bc + 128], ident)
                        yst = pbs.tile([128, 128], F32R, tag="yst")
                        nc.any.tensor_copy(out=yst, in_=pt)
                        nc.sync.dma_start(
                            out=yts[bc][:, 128 * (so - 1):128 * (so - 1) + 128],
                            in_=yst)

            # ---------- phase C: conv transpose ----------
            with tc.tile_pool(name="pc1", bufs=1) as pc1, \
                 tc.tile_pool(name="pc2", bufs=2) as pc2, \
                 tc.tile_pool(name="pcps", bufs=6, space="PSUM") as pcps:
                ysb = pc1.tile([128, 18, 1152], F32R)
                nc.sync.dma_start(out=ysb, in_=yts.rearrange("t b a -> b t a"))
                slab = pc1.tile([128, 50 * 96], F32)
                nc.vector.memset(slab, 0.0)
                for tap in range(16):
                    ky, kx = tap // 4, tap % 4
                    raws = pc2.tile([128, 18, 128], F32R, tag="raws")
                    nc.sync.dma_start(out=raws, in_=rawt_d[tap].rearrange("t b c -> b t c"))
                    fw0 = 1 if kx == 0 else 0
                    fw1 = 47 if kx == 3 else 48
                    nfw = fw1 - fw0
                    for pc in range(3):
                        acc = pcps.tile([128, 384], F32, tag="ct")
                        for bc in range(18):
                            nc.tensor.matmul(out=acc, lhsT=raws[:, bc, :],
                                             rhs=ysb[:, bc, 384 * pc:384 * pc + 384],
                                             start=(bc == 0), stop=(bc == 17))
                        dst = bass.AP(tensor=slab.tensor, offset=slab.offset
                                      + ((16 * pc + ky) * 96 + (kx - 1) + 2 * fw0),
                                      ap=[slab.ap[0], [192, 8], [2, nfw]])
                        src = bass.AP(tensor=acc.tensor, offset=acc.offset + fw0,
                                      ap=[acc.ap[0], [48, 8], [1, nfw]])
                        nc.vector.tensor_add(dst, dst, src)
                nc.sync.dma_start(out=ysl_d, in_=slab)

    nc.compile()
    _CACHED["nc"] = nc
    return nc


def kernel(f, b, mask):
    f = np.asarray(f, np.float32)
    b = np.asarray(b, np.float32)
    mask = np.asarray(mask, np.float32)
    bs = f.shape[0]
    nc = _build_program()
    in_maps = []
    metas = []
    for s in range(bs):
        wnt, mm, rawt, fdp = _prep_sample(f[s], b[s], mask[s])
        mm10 = (mm * np.float32(SCALE)).astype(np.float32)[None, :]
        for h in (0, 1):
            wl = _wnt_window(wnt, (-1 + 9 * h) * 128, 1408)
            wf = _wnt_window(wnt, 2176 if h == 0 else 0, 128)
            g = np.zeros((128, 4), np.float32)
            g[:, 0] = 1.0 - h
            g[:, 1] = h
            g[:, 2] = h
            g[:, 3] = 1.0 - h
            in_maps.append(dict(wl=wl, wf=wf, fdp=fdp, mm10=mm10,
                                mmv=mm[None, :].astype(np.float32),
                                gates=g, rawt=rawt))
            metas.append((s, h))
    r = run_bass_kernel_spmd(nc, in_maps, list(range(8)))
    _CACHED["last"] = (nc, in_maps)

    y = np.zeros((bs, CS, 96, 96), np.float32)
    off_flat = np.zeros((bs, L), np.int64)
    for ci, (s, h) in enumerate(metas):
        res = r.results[ci]
        slab = res["ysl"].reshape(CS, 50, 96)
        if h == 0:
            y[s][:, 0:49, :] += slab[:, 1:50, :]
        else:
            y[s][:, 47:96, :] += slab[:, 0:49, :]
        idx = res["idx"][:, :, 0].astype(np.int64).reshape(9 * 128)
        off_flat[s, 1152 * h:1152 * h + 1152] = idx
    gi = np.arange(IH)[:, None] * np.ones((1, IW), np.int64)
    gj = np.ones((IH, 1), np.int64) * np.arange(IW)[None, :]
    offsets = np.stack([off_flat // 96 - gi.reshape(-1)[None, :],
                        off_flat % 96 - gj.reshape(-1)[None, :]], axis=1)
    offsets = offsets.reshape(bs, 2, IH, IW).astype(np.int32)
    return y, offsets
